# revision 1
# baseline (speedup 1.0000x reference)
"""Causal single-head attention (B=4, S=4096, D=1024, d_key=64) on 8 trn2 cores.

Sharding: 8 cores = 4 batches x 2 key-halves. Core (b, h) handles batch b,
ALL 4096 query rows, and the 16 alternating 128-key blocks {2j+h : j=0..15}.
Each core computes partial PV numerators and softmax denominators over its
key half; the host merges the two halves per batch:
    out = (num_0 + num_1) / (den_0 + den_1).

This halves the K/V HBM traffic per core vs replicating K/V on both cores
of a pair (17.8MB vs 20.6MB) and makes the two cores of a pair perfectly
symmetric (identical work; only the boundary mask data differs by h).

Device kernel (identical SPMD program; per-core differences are input data):
  1. Projections (bf16 matmuls, fp32 accumulate, bf16 results):
     qT [64, 4096] over all rows (direct transposed layout: its consumers
     chain tightly behind it in sweep 0, so extra handoff latency is not
     worth fewer PE cycles), kT [64, 2048] over own key blocks via
     natural-layout matmuls + PE transposes (2.2x fewer PE cycles; the
     latency hides behind the k-quad DMA arrival its consumers stall on
     anyway), and v-natural [128, 65] blocks (data chunk as lhsT -> PV
     lhsT layout directly; column 64 is memset to 1.0 so PV also emits
     the softmax denominator row).
  2. KEY-MAJOR attention sweeps: local key quad t (4 own 128-key blocks =
     512 keys) serves chunks 4t..15, so the work unlocked by each k/v
     arrival shrinks over the DMA stream. Per (chunk, quad) strip: scores
     sT[j, i] = k_j . q_i into a [128, <=1024] PSUM strip, one ACT exp
     (scale=1/8, bf16 out), causal boundary mask multiply on the diagonal
     block only (host-built [128, 256] bf16 triangle, offset by h), then a
     per-strip PV PSUM chain DVE-accumulated into the per-chunk bf16 SBUF
     row; chunk stores go out via the gpsimd DGE so they don't block input
     stage loads on the SP sequencer. A 3-deep pending queue emits each PV
     chain behind later strips' scores so the in-order PE does not wait
     on the ACT exp it just issued.
  3. Input DMA order trickles q groups under sweep 0 and defers k/v quads
     1-3 to the stream tail, matching each sweep's shrinking work.
"""

import numpy as np

import concourse.mybir as mybir
import concourse.tile as tile
from concourse import bacc
from concourse.bass_utils import run_bass_kernel_spmd

B, S, D, DK = 4, 4096, 1024, 64
NCORES = 8
CH = 256  # query rows per chunk
NCH = 16  # chunks per core (all 4096 rows)
KB = 128  # key block
NKB = 16  # own key blocks per core (half of 32)
NKQ = 4  # own key quads (4 blocks of 128 = 512 keys each)
DC = D // 128  # 8 contraction chunks
F32 = mybir.dt.float32
BF16 = mybir.dt.bfloat16

_prog_cache = {}
_last_in_maps = None


def _build(variant):
    assert variant == "causal"

    nc = bacc.Bacc("TRN2", target_bir_lowering=False, debug=False,
                   num_devices=NCORES)

    qt_d = nc.declare_dram_parameter("qt", [D, S], BF16, isOutput=False)
    kt_d = nc.declare_dram_parameter("kt", [D, NKB * KB], BF16, isOutput=False)
    vt_d = nc.declare_dram_parameter("vt", [D, NKB * KB], BF16, isOutput=False)
    # weights packed host-side as [128, DC, DK] so each partition row is
    # contiguous (fast DMA)
    wq_d = nc.declare_dram_parameter("wq", [128, DC * DK], BF16, isOutput=False)
    wk_d = nc.declare_dram_parameter("wk", [128, DC * DK], BF16, isOutput=False)
    wv_d = nc.declare_dram_parameter("wv", [128, DC * DK], BF16, isOutput=False)
    mask_d = nc.declare_dram_parameter("maskq", [KB, CH], BF16, isOutput=False)
    ident_d = nc.declare_dram_parameter("ident", [128, 128], BF16,
                                        isOutput=False)
    # partial numerators (rows 0..63) + denominator (row 64), bf16
    out_d = nc.declare_dram_parameter("out", [NCH, DK + 1, CH], BF16,
                                      isOutput=True)

    qt3 = qt_d.rearrange("(o p) s -> p o s", p=128)
    kt3 = kt_d.rearrange("(o p) s -> p o s", p=128)
    vt3 = vt_d.rearrange("(o p) s -> p o s", p=128)

    NQG = S // 512  # 8 q projection groups of 512 columns
    NKG = NKQ  # 4 k/v projection groups of 512 local keys

    with tile.TileContext(nc) as tc:
        with (
            tc.tile_pool(name="const", bufs=1) as const,
            tc.tile_pool(name="res", bufs=1) as res,
            tc.tile_pool(name="stage", bufs=12) as stage,
            tc.tile_pool(name="natp", bufs=2) as natp,
            tc.tile_pool(name="pwork", bufs=7) as pwork,
            tc.tile_pool(name="ps_mm", bufs=2, space="PSUM") as ps_mm,
            tc.tile_pool(name="ps_s", bufs=2, space="PSUM") as ps_s,
            tc.tile_pool(name="ps_ot", bufs=2, space="PSUM") as ps_ot,
        ):
            def stage_load(src3, sc, splits=4):
                """Split-group DMAs so dependent matmuls can start early."""
                w = DC // splits
                sts = []
                for hh in range(splits):
                    st = stage.tile([128, w, 512], BF16, tag="stage",
                                    name=f"st{hh}")
                    nc.sync.dma_start(
                        st[:],
                        src3[:, w * hh:w * (hh + 1), sc * 512:(sc + 1) * 512])
                    sts.append(st)
                return sts

            def project_kq(w_sb, dst, sc, sts, via_t=False):
                """One 512-column group -> dst tile [64, 512] (bf16).

                via_t=True projects into natural [seq, 64] blocks (data
                chunk as lhsT, N=64 per matmul) then PE-transposes (N=128)
                -- 2.2x fewer PE cycles than the direct N=512 contraction
                steps, but two extra DVE hops of latency. Used for K (its
                consumers stall on the k-quad DMA arrival anyway); Q stays
                direct because sweep-0 strips chain tightly behind it."""
                w = DC // len(sts)
                if not via_t:
                    ps = ps_mm.tile([DK, 512], F32, tag="mm")
                    for dc in range(DC):
                        nc.tensor.matmul(ps[:], w_sb[:, dc, :],
                                         sts[dc // w][:, dc % w, :],
                                         start=(dc == 0), stop=(dc == DC - 1))
                    nc.vector.tensor_copy(dst[:], ps[:])
                    return
                ps = ps_mm.tile([128, 4, DK], F32, tag="mm", name="ps_nat")
                for sb in range(4):
                    for dc in range(DC):
                        nc.tensor.matmul(
                            ps[:, sb, :],
                            sts[dc // w][:, dc % w,
                                         sb * 128:(sb + 1) * 128],
                            w_sb[:, dc, :],
                            start=(dc == 0), stop=(dc == DC - 1))
                nat = natp.tile([128, 4, DK], BF16, tag="nat")
                nc.vector.tensor_copy(nat[:], ps[:])
                pt = ps_mm.tile([DK, 512], BF16, tag="mm", name="ps_t")
                for sb in range(4):
                    nc.tensor.matmul(pt[:, sb * 128:(sb + 1) * 128],
                                     nat[:, sb, :], ident_sb[:],
                                     start=True, stop=True,
                                     is_transpose=True)
                nc.vector.tensor_copy(dst[:], pt[:])

            def project_v(sc, sts):
                """V projected directly to natural [s, c] blocks: lhsT is the
                staged data chunk, rhs the weights -> out [128 s, 64 c], the
                PV lhsT layout (no transposes)."""
                w = DC // len(sts)
                ps = ps_mm.tile([128, 4, DK], F32, tag="mm", name="ps_v")
                for sb in range(4):
                    for dc in range(DC):
                        nc.tensor.matmul(
                            ps[:, sb, :],
                            sts[dc // w][:, dc % w,
                                         sb * 128:(sb + 1) * 128],
                            wv_sb[:, dc, :],
                            start=(dc == 0), stop=(dc == DC - 1))
                for sb in range(4):
                    nc.vector.tensor_copy(vgs[sc][:, sb, 0:DK], ps[:, sb, :])

            # PE warm-up in the initial DMA shadow: keeps the HAM clock at
            # full rate when the first real projections arrive. The short
            # tail of N=256 fills bridge the gap until the first stage DMA
            # lands so the p-state ramp never resets.
            warm = const.tile([128, 512], BF16, tag="warm")
            nc.vector.memset(warm[:], 0.0)
            for _ in range(8):
                wps = ps_mm.tile([DK, 512], F32, tag="mm", name="wps")
                nc.tensor.matmul(wps[:], warm[:, 0:DK], warm[:],
                                 start=True, stop=True)

            wq_sb = const.tile([128, DC, DK], BF16, tag="wq")
            wk_sb = const.tile([128, DC, DK], BF16, tag="wk")
            wv_sb = const.tile([128, DC, DK], BF16, tag="wv")
            nc.sync.dma_start(wq_sb[:], wq_d.rearrange("p (o c) -> p o c", c=DK))
            nc.sync.dma_start(wk_sb[:], wk_d.rearrange("p (o c) -> p o c", c=DK))
            nc.sync.dma_start(wv_sb[:], wv_d.rearrange("p (o c) -> p o c", c=DK))
            msk_sb = const.tile([KB, CH], BF16, tag="msk")
            nc.sync.dma_start(msk_sb[:], mask_d[:])
            ident_sb = const.tile([128, 128], BF16, tag="ident")
            nc.sync.dma_start(ident_sb[:], ident_d[:])

            # kT tiles [64, 512] per local key quad (bf16)
            kts = [res.tile([DK, 512], BF16, tag=f"kt{t}", name=f"kt{t}")
                   for t in range(NKQ)]
            # qT tiles [64, 512] per q group (bf16)
            qts = [res.tile([DK, 512], BF16, tag=f"qt{g}", name=f"qt{g}")
                   for g in range(NQG)]
            # v natural (+ones col): per quad, 4 blocks of [128, 65] (bf16)
            vgs = [res.tile([128, 4, DK + 1], BF16, tag=f"vg{t}",
                            name=f"vg{t}")
                   for t in range(NKQ)]
            for t in range(NKQ):
                nc.vector.memset(vgs[t][:, :, DK:DK + 1], 1.0)

            # bf16 output bounce (PSUM -> SBUF -> DRAM)
            osb = res.tile([DK + 1, NCH, CH], BF16, tag="osb")

            # Input DMA order: the key-major sweep over quad t unlocks
            # (16-4t) chunks of attention, so work-per-arrival decreases over
            # the stream; q groups trickle in under sweep 0.
            load_order = [("k", 0), ("q", 0), ("q", 1), ("v", 0), ("q", 2),
                          ("q", 3), ("q", 4), ("q", 5), ("q", 6), ("q", 7),
                          ("k", 1), ("v", 1), ("k", 2), ("v", 2), ("k", 3),
                          ("v", 3)]
            staged = {}
            emitted = []

            def ensure_loaded(upto):
                """Emit stage loads in order through index `upto`."""
                for i in range(len(emitted), upto + 1):
                    kind, idx = load_order[i]
                    src = {"k": kt3, "q": qt3, "v": vt3}[kind]
                    staged[(kind, idx)] = stage_load(src, idx)
                    emitted.append((kind, idx))

            def q_rhs(c):
                return qts[c // 2][:, (c % 2) * CH:(c % 2 + 1) * CH]

            projected = set()

            def ensure_projected(kind, idx):
                if (kind, idx) in projected:
                    return
                projected.add((kind, idx))
                i = load_order.index((kind, idx))
                ensure_loaded(i)
                sts = staged.pop((kind, idx))
                if kind == "q":
                    project_kq(wq_sb, qts[idx], idx, sts)
                elif kind == "k":
                    project_kq(wk_sb, kts[idx], idx, sts, via_t=True)
                else:
                    project_v(idx, sts)

            # prefetch schedule: keep a few loads in flight ahead of compute
            def prefetch(upto):
                ensure_loaded(min(upto, len(load_order) - 1))

            # Strip pipeline with lag: emit scores+exp for strip N+LAG before
            # the PV chain of strip N, so the in-order PE never waits on the
            # ACT exp of the strip it just produced.
            pending = []  # dicts awaiting PV emission

            def emit_pv(item):
                c, t, nb = item["c"], item["t"], item["nb"]
                ensure_projected("v", t)
                o_ps = ps_ot.tile([DK + 1, CH], F32, tag="ot", name="o_ps")
                for u in range(nb):
                    nc.tensor.matmul(
                        o_ps[:], vgs[t][:, u, :],
                        item["p"][:, u * CH:(u + 1) * CH],
                        start=(u == 0), stop=(u == nb - 1))
                if t == 0:
                    nc.vector.tensor_copy(osb[:, c, :], o_ps[:])
                else:
                    nc.vector.tensor_add(osb[:, c, :], osb[:, c, :], o_ps[:])
                if t == c // 4:  # chunk finished: store partials
                    eng = nc.sync if t == NKQ - 1 else nc.gpsimd
                    eng.dma_start(out_d[c], osb[:, c, :])

            def drain(upto):
                while len(pending) > upto:
                    emit_pv(pending.pop(0))

            def strip(c, t):
                """Scores + exp (+ boundary mask) for quad t of chunk c."""
                nb = 4 if t < c // 4 else c % 4 + 1
                ncols = nb * CH
                s_ps = ps_s.tile([KB, 4 * CH], F32, tag="s", name="s_ps")
                for u in range(nb):
                    nc.tensor.matmul(
                        s_ps[:, u * CH:(u + 1) * CH],
                        kts[t][:, u * KB:(u + 1) * KB],
                        q_rhs(c), start=True, stop=True)
                p_sb = pwork.tile([KB, 4 * CH], BF16, tag="p")
                nc.scalar.activation(
                    p_sb[:, 0:ncols], s_ps[:, 0:ncols],
                    mybir.ActivationFunctionType.Exp, scale=0.125)
                if t == c // 4:
                    # causal boundary: diagonal block is the last one
                    sl = slice((nb - 1) * CH, nb * CH)
                    nc.vector.tensor_mul(p_sb[:, sl], p_sb[:, sl], msk_sb[:])
                pending.append(dict(c=c, t=t, nb=nb, p=p_sb))

            # Key-major sweeps over chunks 4-15: quad t serves chunks
            # max(4,4t)..15, so the work unlocked by each k/v arrival
            # shrinks over the stream. Sweep 0 interleaves the q-group
            # projections as their loads land.
            for t in range(NKQ):
                if t > 0:
                    # flush pending PVs before the PE stalls on the next
                    # k-quad projection (their inputs are already on chip)
                    drain(0)
                ensure_projected("k", t)
                for c in range(4 * t, NCH):
                    # pop a pending PV BEFORE this strip's scores: if the
                    # scores stall on a PSUM bank (waiting an exp), the PE
                    # does useful PV work first instead of queueing it
                    # behind the stall
                    drain(3)
                    if t == 0:
                        ensure_projected("q", c // 2)
                        prefetch(c + 4)
                    strip(c, t)
            drain(0)

    nc.compile()
    return nc


def _get_prog(variant):
    if variant not in _prog_cache:
        _prog_cache[variant] = _build(variant)
    return _prog_cache[variant]


def _mask_block(h):
    """Multiplicative boundary mask [KB, CH] for the diagonal own-block of
    every chunk of core-half h: local key row kappa (global key 256c + 128h
    + kappa) allows query column i (global row 256c + i) iff
    i >= kappa + 128h."""
    i = np.arange(CH)[None, :]
    kap = np.arange(KB)[:, None]
    return (i >= kap + 128 * h).astype(np.float32)


def kernel(queries, keys, values, Wq, Wk, Wv, mask):
    import ml_dtypes  # noqa: F401  registers numpy bfloat16

    bf16 = np.dtype("bfloat16")
    queries = np.asarray(queries, dtype=np.float32)
    keys = np.asarray(keys, dtype=np.float32)
    values = np.asarray(values, dtype=np.float32)
    mask_np = np.asarray(mask)

    causal = bool(np.array_equal(
        mask_np != 0, np.tril(np.ones((S, S), dtype=bool))))
    if not causal:
        raise NotImplementedError("only the causal mask is supported")

    qt = np.ascontiguousarray(queries.transpose(0, 2, 1)).astype(bf16)
    # per-batch [D, S] -> own-key-half [D, 2048] with alternating 128 blocks
    kt_f = np.asarray(keys, dtype=np.float32).transpose(0, 2, 1)
    vt_f = np.asarray(values, dtype=np.float32).transpose(0, 2, 1)
    kt_blk = kt_f.reshape(B, D, S // KB, KB)
    vt_blk = vt_f.reshape(B, D, S // KB, KB)

    def pack_w(W):
        # [DK, D] -> [128, DC*DK] with w[p, dc*DK+k] = W[k, dc*128+p]
        Wt = np.asarray(W, dtype=np.float32).T.reshape(DC, 128, DK)
        return np.ascontiguousarray(Wt.transpose(1, 0, 2).reshape(128, DC * DK)
                                    ).astype(bf16)

    wq, wk, wv = pack_w(Wq), pack_w(Wk), pack_w(Wv)
    ident = np.eye(128, dtype=np.float32).astype(bf16)

    in_maps = []
    for core in range(NCORES):
        b, h = divmod(core, 2)
        kth = np.ascontiguousarray(
            kt_blk[b, :, h::2, :].reshape(D, NKB * KB)).astype(bf16)
        vth = np.ascontiguousarray(
            vt_blk[b, :, h::2, :].reshape(D, NKB * KB)).astype(bf16)
        m = {"qt": qt[b], "kt": kth, "vt": vth,
             "wq": wq, "wk": wk, "wv": wv, "ident": ident,
             "maskq": _mask_block(h).astype(bf16)}
        in_maps.append(m)

    global _last_in_maps
    _last_in_maps = in_maps
    nc = _get_prog("causal")
    res = run_bass_kernel_spmd(nc, in_maps, list(range(NCORES)))

    out = np.empty((B, S, DK), dtype=np.float32)
    ov = out.reshape(B, NCH, CH, DK)
    for b in range(B):
        r0 = np.asarray(res.results[2 * b]["out"], dtype=np.float32)
        r1 = np.asarray(res.results[2 * b + 1]["out"], dtype=np.float32)
        num = r0[:, :DK, :] + r1[:, :DK, :]  # [NCH, DK, CH]
        den = r0[:, DK:DK + 1, :] + r1[:, DK:DK + 1, :]  # [NCH, 1, CH]
        ov[b] = (num / den).transpose(0, 2, 1)
    return out


if __name__ == "__main__":
    rng = np.random.default_rng(0)
    q = rng.standard_normal((B, S, D), dtype=np.float32)
    k = rng.standard_normal((B, S, D), dtype=np.float32)
    v = rng.standard_normal((B, S, D), dtype=np.float32)
    sc = 1.0 / np.sqrt(D)
    wq = rng.uniform(-sc, sc, (DK, D)).astype(np.float32)
    wk = rng.uniform(-sc, sc, (DK, D)).astype(np.float32)
    wv = rng.uniform(-sc, sc, (DK, D)).astype(np.float32)
    msk = np.tril(np.ones((S, S), dtype=np.int32))
    out = kernel(queries=q, keys=k, values=v, Wq=wq, Wk=wk, Wv=wv, mask=msk)
    print("out", out.shape, out.dtype, float(np.abs(out).mean()))

